# revision 1
# baseline (speedup 1.0000x reference)
"""ASGCN unit kernel for 8 Trainium2 NeuronCores (data-parallel over batch).

Contract: kernel(**inputs) takes the FULL unsharded inputs and returns the
FULL [128, 256] float32 output. Batch is sharded 16 samples/core across 8
cores; all parameters are replicated.

Algorithm notes (vs the reference):
  - position_weight and the aspect mask are affine in the int length tensors;
    both are precomputed on host. The position weight w AND the degree
    normalization 1/(deg+1) are folded into the transposed adjacency:
        adj @ diag(w) @ X / den == (adjT * w[t] * dinv[s]).T @ X
  - w[t] == 0 for t >= text_len, so whole 128-wide chunks of the weighted
    adjacency are structurally zero. Samples are sorted by
    n = ceil(text_len/128) and dealt into per-core slots so that all 8 cores
    share one slot->n pattern (SPMD); matmuls/DMAs skip the zero chunks.
  - the aspect mask keeps only rows [left_len, left_len+aspect_len) (<= 5
    rows) of layer 2's output, so layer 2 is reassociated:
        relu(adj_win_wd @ x2 @ W2 + b2)
    which needs only a [WIN, S] slice of adj and a [WIN, D] @ [D, D] matmul.
  - GCN matmuls (phases 1-3) run in fp8e4 (e4m3, +-240) with DoubleRow
    perf mode where chunk pairs allow (two 128-K-tiles per instruction,
    2 cols/cycle). Power-of-two scales keep everything in fp8's normal
    range and are folded into host constants / activation scale params:
        W1 *= 16, adjTw *= 4096, adjWTw *= 4096, W2 *= 64,
        b1 *= 2^16 (bf16 bias matmul into PSUM), b2 via per-partition
        activation bias, x2 carries 2^8, yT carries 2^6.
    Validated host-side at ~6e-3 max relative error (budget 2e-2).
  - attention (phase 4) stays bf16 (fp8 logits or fp8 weighted-sum fail the
    error budget): logits via M=1 matmuls, exp on the scalar engine with
    sum(exp) as a free accumulator output, p broadcast to 128 partitions on
    the gpsimd engine, weighted sum as a vector STT with free-axis
    accumulation. Softmax needs no max-subtraction (logits are O(5));
    normalization happens on host from the returned sum(exp).
"""

import sys

if "/opt/trn_rl_repo" not in sys.path:
    sys.path.insert(0, "/opt/trn_rl_repo")

import numpy as np
import ml_dtypes

B, S, D, WIN = 128, 512, 256, 8
NCORES = 8
BPC = B // NCORES  # samples per core
GRP = 2
BF = ml_dtypes.bfloat16
F8 = ml_dtypes.float8_e4m3  # TRN fp8e4: max +-240

_nc_cache = {}


def _build_nc(bpc, n_slots):
    """n_slots[b] = number of active 128-chunks (2..4) for slot b."""
    from contextlib import ExitStack

    import concourse.bass as bass
    import concourse.tile as tile
    from concourse import bacc, mybir

    dt = mybir.dt
    f32, bf16, f8 = dt.float32, dt.bfloat16, dt.float8e4
    AF = mybir.ActivationFunctionType
    OP = mybir.AluOpType
    DR = mybir.MatmulPerfMode.DoubleRow
    ts = bass.ts

    nc = bacc.Bacc("TRN2", target_bir_lowering=False, debug=False,
                   num_devices=NCORES)

    # --- DRAM parameters (per-core shard; layouts match SBUF tiles) ---
    axt8_d = nc.declare_dram_parameter("axt8", [bpc, 128, 2 * S], f8,
                                       isOutput=False)
    adj8_d = nc.declare_dram_parameter("adj8", [bpc, 128, 4 * S], f8,
                                       isOutput=False)
    awm_d = nc.declare_dram_parameter("awm", [bpc, 128, 4 * WIN], f8,
                                      isOutput=False)
    mw_d = nc.declare_dram_parameter("mw", [bpc, 128, WIN], bf16,
                                     isOutput=False)
    axtb_d = nc.declare_dram_parameter("axtb", [bpc, 128, 2 * S], bf16,
                                       isOutput=False)
    W1_d = nc.declare_dram_parameter("W1s8", [128, 2, D], f8, isOutput=False)
    W2_d = nc.declare_dram_parameter("W2s8", [128, 2, D], f8, isOutput=False)
    b1_d = nc.declare_dram_parameter("b1B", [128, 2 * D], bf16, isOutput=False)
    b2_d = nc.declare_dram_parameter("b2col", [128, 2], f32, isOutput=False)
    outU_d = nc.declare_dram_parameter("outU", [128, 2 * bpc], f32,
                                       isOutput=True)
    sume_d = nc.declare_dram_parameter("sume", [1, bpc], f32, isOutput=True)

    with tile.TileContext(nc) as tc, ExitStack() as ctx:
        const = ctx.enter_context(tc.tile_pool(name="const", bufs=1))
        pxt = ctx.enter_context(tc.tile_pool(name="pxt", bufs=8))
        padj = ctx.enter_context(tc.tile_pool(name="padj", bufs=10))
        pxb = ctx.enter_context(tc.tile_pool(name="pxb", bufs=12))
        psm = ctx.enter_context(tc.tile_pool(name="psm", bufs=10))
        pmid = ctx.enter_context(tc.tile_pool(name="pmid", bufs=12))
        pscr = ctx.enter_context(tc.tile_pool(name="pscr", bufs=4))
        psmall = ctx.enter_context(tc.tile_pool(name="psmall", bufs=12))
        pstage = ctx.enter_context(tc.tile_pool(name="pstage", bufs=1))
        psH = ctx.enter_context(tc.tile_pool(name="psH", bufs=2, space="PSUM"))
        psG = ctx.enter_context(tc.tile_pool(name="psG", bufs=2, space="PSUM"))
        psS = ctx.enter_context(tc.tile_pool(name="psS", bufs=2, space="PSUM"))
        psl = ctx.enter_context(tc.tile_pool(name="psl", bufs=1, space="PSUM"))
        psP = ctx.enter_context(tc.tile_pool(name="psP", bufs=1, space="PSUM"))

        # W1 first: phase 1 of group 0 needs only it + axt8
        W1s8 = const.tile([128, 2, D], f8, tag="W1s8")
        nc.sync.dma_start(W1s8[:], W1_d[:])
        ones1 = const.tile([1, 128], bf16, tag="ones1")
        nc.vector.memset(ones1[:], 1.0)

        outU = pstage.tile([128, 2 * bpc], f32, tag="outU")
        sume = pstage.tile([1, bpc], f32, tag="sume")

        deferred = {}

        def emit_late_consts():
            W2s8 = const.tile([128, 2, D], f8, tag="W2s8")
            nc.sync.dma_start(W2s8[:], W2_d[:])
            b1B = const.tile([128, 2, D], bf16, tag="b1B")
            nc.sync.dma_start(b1B[:], b1_d[:].rearrange("p (c d) -> p c d", d=D))
            b2col = const.tile([128, 2], f32, tag="b2col")
            nc.sync.dma_start(b2col[:], b2_d[:])
            deferred.update(W2s8=W2s8, b1B=b1B, b2col=b2col)

        def emit_dma_crit(grp, T):
            # phase-1/2 critical data, sized to the slot's chunk count
            for b in grp:
                n = n_slots[b]
                axt8 = pxt.tile([128, 2, S], f8, name="axt8", tag="axt8")
                nc.sync.dma_start(axt8[:, :, 0:128 * n],
                                  axt8_d[b].rearrange(
                                      "p (c s) -> p c s", s=S)[:, :, 0:128 * n])
                T.setdefault(b, {})["axt8"] = axt8
            for b in grp:
                n = n_slots[b]
                adj8 = padj.tile([128, 4, S], f8, name="adj8", tag="adj8")
                nc.sync.dma_start(adj8[:, 0:n, :],
                                  adj8_d[b].rearrange(
                                      "p (c s) -> p c s", s=S)[:, 0:n, :])
                T[b]["adj8"] = adj8

        def emit_dma_lazy(grp, T):
            # phase-3/4 data: window adjacency, mask, bf16 attention copy
            for b in grp:
                n = n_slots[b]
                awm = psm.tile([128, 4, WIN], f8, name="awm", tag="awm")
                nc.sync.dma_start(awm[:, 0:n, :],
                                  awm_d[b].rearrange(
                                      "p (c w) -> p c w", w=WIN)[:, 0:n, :])
                mw = psm.tile([128, WIN], bf16, name="mw", tag="mw")
                nc.sync.dma_start(mw[:], mw_d[b])
                T.setdefault(b, {})["awm"] = awm
                T[b]["mw"] = mw
            for b in grp:
                axtb = pxb.tile([128, 2, S], bf16, name="axtb", tag="axtb")
                nc.sync.dma_start(axtb[:], axtb_d[b].rearrange(
                    "p (c s) -> p c s", s=S))
                T[b]["axtb"] = axtb

        def emit_p1(grp, T):
            # h1[s,e] = sum_d x[s,d] W1[d,e]  (fp8 DoubleRow over the 2
            # d-chunks; PSUM holds 16*h1 from the W1 prescale)
            for b in grp:
                n = n_slots[b]
                axt8 = T[b]["axt8"]
                h1s8 = pmid.tile([128, 4, D], f8, name="h1s8", tag="h1s8")
                for sc in range(n):
                    ps_h = psH.tile([128, D], f32, name="ps_h", tag="ps_h")
                    for dc in range(2):
                        nc.tensor.matmul(ps_h[:], axt8[:, dc, ts(sc, 128)],
                                         W1s8[:, dc, :],
                                         start=(dc == 0), stop=(dc == 1))
                    # PSUM -> SBUF fp8 (value 16*h1); split across engines
                    if sc % 2 == 0:
                        nc.scalar.copy(h1s8[:, sc, :], ps_h[:])
                    else:
                        nc.vector.tensor_copy(h1s8[:, sc, :], ps_h[:])
                T[b]["h1s8"] = h1s8

        def emit_p2(grp, T):
            # g1 = b1 + adjTw.T @ h1 ; x2 = relu(g1) * 2^8 in fp8
            # PSUM = 2^16 * (g1 - b1) + 2^16 * b1 (bf16 bias matmul)
            b1B = deferred["b1B"]
            for b in grp:
                n = n_slots[b]
                adj8, h1s8 = T[b]["adj8"], T[b]["h1s8"]
                x2 = pmid.tile([128, 4, D], f8, name="x2", tag="x2")
                for half in range((n + 1) // 2):
                    w_ = min(2, n - 2 * half)
                    ps_g = psG.tile([128, 2, D], f32, name="ps_g", tag="ps_g")
                    for sci in range(w_):
                        sc = 2 * half + sci
                        last = (sci == w_ - 1)
                        for tc_ in range(n):
                            nc.tensor.matmul(
                                ps_g[:, sci, :],
                                adj8[:, tc_, ts(sc, 128)],
                                h1s8[:, tc_, :],
                                start=(tc_ == 0),
                                stop=(last and tc_ == n - 1))
                    # g1 = 2^-16 psum + b1 (vector), then relu*2^8 -> fp8
                    # on the idle gpsimd engine
                    gt = pmid.tile([128, 2, D], bf16, name="gt", tag="gt")
                    nc.vector.scalar_tensor_tensor(
                        gt[:, 0:w_, :], ps_g[:, 0:w_, :], 1.0 / 65536.0,
                        b1B[:, 0:w_, :], op0=OP.mult, op1=OP.add)
                    if half == 0:
                        nc.scalar.activation(
                            x2[:, 0:w_, :], gt[:, 0:w_, :],
                            AF.Relu, scale=256.0)
                    else:
                        nc.vector.tensor_scalar(
                            x2[:, 2:2 + w_, :], gt[:, 0:w_, :],
                            256.0, 0.0, op0=OP.mult, op1=OP.max)
                T[b]["x2"] = x2

        def emit_p3a(grp, T):
            # window layer part a: yT[d, win] = sum_s x2[s,d] awm[s,win]
            # (PSUM = 2^20 y), then fp8 copy carrying 2^6
            for b in grp:
                n = n_slots[b]
                awm, x2 = T[b]["awm"], T[b]["x2"]
                ps_y = psS.tile([128, 2, WIN], f32, name="ps_y", tag="ps_s")
                for dc in range(2):
                    for sc in range(n):
                        nc.tensor.matmul(ps_y[:, dc, :],
                                         x2[:, sc, ts(dc, 128)],
                                         awm[:, sc, :],
                                         start=(sc == 0), stop=(sc == n - 1))
                yT8 = psmall.tile([128, 2, WIN], f8, name="yT8", tag="yT8")
                nc.vector.tensor_scalar(yT8[:], ps_y[:], 1.0 / 16384.0, 0.0,
                                        op0=OP.mult, op1=OP.add)
                T[b]["yT8"] = yT8

        def emit_p3b(grp, T):
            # part b: z.T[e, win] = sum_d W2[d,e] yT[d,win]  (PSUM = 2^12 z),
            # r1 = relu(2^-12 ps_z + b2[e]), xsum over masked win -> xsb bf16
            W2s8, b2col = deferred["W2s8"], deferred["b2col"]
            for b in grp:
                yT8, mw = T[b]["yT8"], T[b]["mw"]
                ps_z = psS.tile([128, 2, WIN], f32, name="ps_z", tag="ps_s")
                for ec in range(2):
                    for dc in range(2):
                        nc.tensor.matmul(ps_z[:, ec, :],
                                         W2s8[:, dc, ts(ec, 128)],
                                         yT8[:, dc, :],
                                         start=(dc == 0), stop=(dc == 1))
                r1 = psmall.tile([128, 2, WIN], f32, name="r1", tag="r1")
                for ec in range(2):
                    nc.scalar.activation(r1[:, ec, :], ps_z[:, ec, :],
                                         AF.Relu, bias=b2col[:, ec:ec + 1],
                                         scale=1.0 / 4096.0)
                xsb = psmall.tile([128, 2], bf16, name="xsb", tag="xsb")
                for ec in range(2):
                    x3 = psmall.tile([128, WIN], f32, name="x3", tag="x3")
                    xs_f = psmall.tile([128, 1], f32, name="xs_f", tag="xs_f")
                    nc.vector.scalar_tensor_tensor(
                        x3[:], r1[:, ec, :], 1.0, mw[:],
                        op0=OP.mult, op1=OP.mult, accum_out=xs_f[:])
                    nc.gpsimd.tensor_copy(xsb[:, ec:ec + 1], xs_f[:])
                T[b]["xsb"] = xsb

        def emit_p4a(grp, T):
            # attention part a (bf16): logits + exp with sum(exp) accumulator
            # (both samples' logits share one PSUM bank at 512-col offsets)
            ps_l = psl.tile([33, S], f32, name="ps_l", tag="ps_l")
            for gi, b in enumerate(grp):
                axtb, xsb = T[b]["axtb"], T[b]["xsb"]
                for dc in range(2):
                    nc.tensor.matmul(ps_l[32 * gi:32 * gi + 1, :],
                                     xsb[:, dc:dc + 1], axtb[:, dc, :],
                                     start=(dc == 0), stop=(dc == 1))
            for gi, b in enumerate(grp):
                p_t = psmall.tile([1, S], bf16, name="p_t", tag="p_t")
                nc.scalar.activation(p_t[:], ps_l[32 * gi:32 * gi + 1, :],
                                     AF.Exp, accum_out=sume[:, b:b + 1])
                T[b]["p_t"] = p_t

        def emit_p4b(grp, T):
            # part b: broadcast p over partitions (K=1 matmul), weighted sum
            for b in grp:
                axtb, p_t = T[b]["axtb"], T[b]["p_t"]
                ps_pb = psP.tile([128, S], f32, name="ps_pb", tag="ps_pb")
                nc.tensor.matmul(ps_pb[:], ones1[:], p_t[:])
                pbb = pscr.tile([128, S], bf16, name="pbb", tag="pbb")
                nc.scalar.copy(pbb[:], ps_pb[:])
                for dc in range(2):
                    scr = pscr.tile([128, S], bf16, name="scr", tag="scr")
                    nc.vector.scalar_tensor_tensor(
                        scr[:], axtb[:, dc, :], 1.0, pbb[:],
                        op0=OP.mult, op1=OP.mult,
                        accum_out=outU[:, 2 * b + dc:2 * b + dc + 1])

        groups = [list(range(g0, min(g0 + GRP, bpc)))
                  for g0 in range(0, bpc, GRP)]
        ngr = len(groups)
        tiles = {gi: {} for gi in range(ngr)}
        # Software-pipeline ladder: every phase lags its producer by a full
        # iteration so no tensor instruction waits on same-iteration
        # scalar/vector chains. DMA queue keeps p1/p2 feeds 2 iterations
        # ahead and attention (bf16) data out of their way.
        emit_dma_crit(groups[0], tiles[0])
        emit_late_consts()
        if ngr > 1:
            emit_dma_crit(groups[1], tiles[1])
        emit_dma_lazy(groups[0], tiles[0])
        for i in range(ngr + 3):
            if i + 2 < ngr:
                emit_dma_crit(groups[i + 2], tiles[i + 2])
            if 0 < i + 1 < ngr:
                emit_dma_lazy(groups[i + 1], tiles[i + 1])
            if 0 <= i - 3 < ngr:
                emit_p4a(groups[i - 3], tiles[i - 3])
            if i < ngr:
                emit_p1(groups[i], tiles[i])
            if 0 <= i - 2 < ngr:
                emit_p3a(groups[i - 2], tiles[i - 2])
            if 0 <= i - 1 < ngr:
                emit_p2(groups[i - 1], tiles[i - 1])
            if 0 <= i - 2 < ngr:
                emit_p3b(groups[i - 2], tiles[i - 2])
            if 0 <= i - 3 < ngr:
                emit_p4b(groups[i - 3], tiles[i - 3])

        nc.sync.dma_start(outU_d[:], outU[:])
        nc.sync.dma_start(sume_d[:], sume[:])

    nc.compile()
    return nc


def _f8(x):
    return np.clip(x, -240.0, 240.0).astype(F8)


def _plan(inputs):
    """Host-side preprocessing: fold position weight / degree norm / fp8
    scales into the shipped tensors; sort samples by chunk count into
    per-core slots. order[b*NCORES + c] is the original sample index
    placed in slot b of core c."""
    text_out = np.asarray(inputs["text_out"], dtype=np.float32)
    adj = np.asarray(inputs["adj"], dtype=np.float32)
    W1 = np.asarray(inputs["W1"], dtype=np.float32)
    b1 = np.asarray(inputs["b1"], dtype=np.float32)
    W2 = np.asarray(inputs["W2"], dtype=np.float32)
    b2 = np.asarray(inputs["b2"], dtype=np.float32)
    tl = np.asarray(inputs["text_len"]).astype(np.int64)
    al = np.asarray(inputs["aspect_len"]).astype(np.int64)
    ll = np.asarray(inputs["left_len"]).astype(np.int64)

    n_all = np.minimum(4, np.maximum(2, (tl + 127) // 128)).astype(np.int64)
    order = np.argsort(n_all, kind="stable")        # [B]
    n_slots = tuple(int(n_all[order[b * NCORES:(b + 1) * NCORES]].max())
                    for b in range(BPC))

    j = np.arange(S)[None, :]
    start = ll[:, None]
    end = (ll + al - 1)[:, None]
    ctxlen = (tl - al).astype(np.float32)[:, None]
    w = np.where(j < start, 1.0 - (start - j) / ctxlen,
                 np.where(j <= end, 0.0,
                          np.where(j < tl[:, None], 1.0 - (j - end) / ctxlen,
                                   0.0))).astype(np.float32)      # [B,S]
    dinv = (1.0 / (adj.sum(axis=2) + 1.0)).astype(np.float32)     # [B,S]

    # transposed adjacency, position weight (t) and 1/den (s) folded, *4096
    adjTw = adj.transpose(0, 2, 1) * (4096.0 * w[:, :, None]) * dinv[:, None, :]
    adj8 = _f8(np.ascontiguousarray(
        adjTw.reshape(B, 4, 128, S).transpose(0, 2, 1, 3)).reshape(B, 128, 4 * S))

    xT = text_out.transpose(0, 2, 1)                               # [B,D,S]
    xT = np.ascontiguousarray(
        xT.reshape(B, 2, 128, S).transpose(0, 2, 1, 3)).reshape(B, 128, 2 * S)
    axt8 = _f8(xT)
    axtb = xT.astype(BF)

    win = np.clip(ll[:, None] + np.arange(WIN)[None, :], 0, S - 1)  # [B,WIN]
    adj_win = np.take_along_axis(adj, win[:, :, None], axis=1)      # [B,WIN,S]
    dinvW = np.take_along_axis(dinv, win, axis=1)                   # [B,WIN]
    adjWTw = (adj_win.transpose(0, 2, 1) * (4096.0 * w[:, :, None])
              * dinvW[:, None, :])
    awm = _f8(np.ascontiguousarray(
        adjWTw.reshape(B, 4, 128, WIN).transpose(0, 2, 1, 3)).reshape(
            B, 128, 4 * WIN))

    mw = np.ascontiguousarray(np.broadcast_to(
        (np.arange(WIN)[None, None, :] < al[:, None, None]), (B, 128, WIN))
    ).astype(BF)

    W1s8 = _f8(np.ascontiguousarray(
        (16.0 * W1).reshape(2, 128, D).transpose(1, 0, 2)))
    W2s8 = _f8(np.ascontiguousarray(
        (64.0 * W2).reshape(2, 128, D).transpose(1, 0, 2)))
    b1B = np.ascontiguousarray(np.broadcast_to(
        np.tile(b1, 2)[None, :], (128, 2 * D))).astype(BF)         # [128, 2D]
    b2col = np.ascontiguousarray(b2.reshape(2, 128).T).astype(np.float32)

    in_maps = []
    for c in range(NCORES):
        idx = order[np.arange(BPC) * NCORES + c]   # slot b -> order[b*8+c]
        in_maps.append({
            "axt8": np.ascontiguousarray(axt8[idx]),
            "adj8": np.ascontiguousarray(adj8[idx]),
            "awm": np.ascontiguousarray(awm[idx]),
            "mw": np.ascontiguousarray(mw[idx]),
            "axtb": np.ascontiguousarray(axtb[idx]),
            "W1s8": W1s8, "W2s8": W2s8, "b1B": b1B, "b2col": b2col,
        })
    return in_maps, n_slots, order


def _assemble(results, order):
    out = np.empty((B, D), dtype=np.float32)
    for c in range(NCORES):
        outU = results[c]["outU"]              # [128, 2*BPC]
        sume = results[c]["sume"].reshape(-1)  # [BPC]
        for b in range(BPC):
            col = outU[:, 2 * b:2 * b + 2]     # [128, 2] (p, dc)
            out[order[b * NCORES + c]] = col.T.reshape(-1) / sume[b]
    return out


def kernel(**inputs):
    from concourse.bass_utils import run_bass_kernel_spmd

    in_maps, n_slots, order = _plan(inputs)
    key = (BPC, n_slots)
    if key not in _nc_cache:
        _nc_cache[key] = _build_nc(BPC, n_slots)
    nc = _nc_cache[key]
    res = run_bass_kernel_spmd(nc, in_maps, list(range(NCORES)))
    return _assemble(res.results, order)

